# revision 10
# baseline (speedup 1.0000x reference)
"""AttnHGCN 2-hop GNN message passing on 8 Trainium2 NeuronCores.

Strategy (edge/graph parallelism):
  - KG edges are bucketed by head (destination) node. Each core owns a
    contiguous range of 25088 entity rows (196 blocks of 128) and receives
    exactly the edges whose head falls in its range.  Within a core, edges
    are grouped per 128-node block and padded to T tiles of 128 edges.
  - Per edge tile: indirect-DMA gathers of entity rows, one-hot matmul for
    relation embedding lookup, fused multiply+reduce for attention logits,
    exp on the scalar engine, and a PE matmul with a data-built 0/1
    selection matrix that performs the per-node segment sum of both the
    exp-weighted messages and the softmax denominators in one PSUM
    accumulation.
  - Per block: divide by the denominator, l2-normalize, accumulate the
    residual.  User/item edges follow the same scheme (no softmax).
  - Hop boundary: AllGather of the normalized entity blocks, then a full
    local pass applying W_Q to rebuild the per-node query table.
"""
import sys
sys.path.insert(0, "/opt/trn_rl_repo")

import math
from contextlib import ExitStack
import numpy as np

from concourse import bass, bacc, mybir, tile
from concourse.bass_utils import run_bass_kernel_spmd
from concourse.masks import make_identity

P = 128
C = 128
H = 2
DK = 64
N_USERS = 100_000
N_ENT = 200_000
N_REL = 9            # used relations (edge_type-1 in [0, 9))
E_KG = 1_000_000
E_UI = 1_000_000
CORES = 8

EBLK = 196           # entity 128-blocks per core
EB = EBLK * P        # 25088 entity rows per core
NEP = CORES * EB     # 200704 padded entity rows
UBLK = 98            # user 128-blocks per core
UB = UBLK * P        # 12544 user rows per core
NUP = CORES * UB

F32 = mybir.dt.float32
F16 = mybir.dt.float16
I32 = mybir.dt.int32
Alu = mybir.AluOpType
Act = mybir.ActivationFunctionType


# ----------------------------------------------------------------- host prep

def _bucketize(dest, ncores, rows_per_core, blocks_per_core):
    """Return (core, global_block, slot_rank, order) for each edge sorted by
    global block, plus per-block counts."""
    core = dest // rows_per_core
    blk = (dest % rows_per_core) // P
    gb = core * blocks_per_core + blk
    order = np.argsort(gb, kind="stable")
    gbs = gb[order]
    nblocks = ncores * blocks_per_core
    counts = np.bincount(gbs, minlength=nblocks)
    starts = np.concatenate([[0], np.cumsum(counts)[:-1]])
    rank = np.arange(len(dest)) - starts[gbs]
    return order, gbs, rank, counts


def _prep_kg(edge_index, edge_type):
    head = np.asarray(edge_index[0], dtype=np.int64)
    tail = np.asarray(edge_index[1], dtype=np.int64)
    et = np.asarray(edge_type, dtype=np.int64) - 1
    order, gbs, rank, counts = _bucketize(head, CORES, EB, EBLK)
    T = max(1, int(math.ceil(counts.max() / P)))
    nslot_blk = T * P
    nslots = CORES * EBLK * nslot_blk
    g_tail = np.zeros(nslots, np.int32)
    g_q = np.zeros(nslots, np.int32)
    g_hr = np.full(nslots, -1.0, np.float32)
    g_oh = np.zeros((N_REL, nslots), np.float32)
    slot = gbs * nslot_blk + rank
    g_tail[slot] = tail[order]
    g_q[slot] = head[order]
    g_hr[slot] = (head[order] % P).astype(np.float32)
    g_oh[et[order], slot] = 1.0
    per_core = EBLK * nslot_blk
    outs = []
    for c in range(CORES):
        sl = slice(c * per_core, (c + 1) * per_core)
        outs.append(dict(
            kg_tail=g_tail[sl].reshape(EBLK * T, P).T.copy(),
            kg_q=g_q[sl].reshape(EBLK * T, P).T.copy(),
            kg_hr=g_hr[sl].reshape(EBLK * T, P).T.astype(np.float16),
            kg_oh=np.ascontiguousarray(g_oh[:, sl]).astype(np.float16),
        ))
    return T, outs


def _prep_ui(inter_edge, inter_w):
    user = np.asarray(inter_edge[0], dtype=np.int64)
    item = np.asarray(inter_edge[1], dtype=np.int64)
    w = np.asarray(inter_w, dtype=np.float32)
    order, gbs, rank, counts = _bucketize(user, CORES, UB, UBLK)
    T = max(1, int(math.ceil(counts.max() / P)))
    nslot_blk = T * P
    nslots = CORES * UBLK * nslot_blk
    g_item = np.zeros(nslots, np.int32)
    g_ur = np.full(nslots, -1.0, np.float32)
    g_w = np.zeros(nslots, np.float32)
    slot = gbs * nslot_blk + rank
    g_item[slot] = item[order]
    g_ur[slot] = (user[order] % P).astype(np.float32)
    g_w[slot] = w[order]
    per_core = UBLK * nslot_blk
    outs = []
    for c in range(CORES):
        sl = slice(c * per_core, (c + 1) * per_core)
        outs.append(dict(
            ui_item=g_item[sl].reshape(UBLK * T, P).T.copy(),
            ui_ur=g_ur[sl].reshape(UBLK * T, P).T.astype(np.float16),
            ui_w=g_w[sl].reshape(UBLK * T, P).T.astype(np.float16),
        ))
    return T, outs


# ------------------------------------------------------------- bass program

def build_program(T, TU):
    nc = bacc.Bacc(num_devices=CORES)

    entv0 = nc.dram_tensor("entv0", [NEP, C], F16, kind="ExternalInput")
    e0T = nc.dram_tensor("e0T", [C, NEP], F16, kind="ExternalInput")
    e0_blk = nc.dram_tensor("e0_blk", [EB, C], F32, kind="ExternalInput")
    u0_blk = nc.dram_tensor("u0_blk", [UB, C], F32, kind="ExternalInput")
    wq = nc.dram_tensor("wq", [C, C], F16, kind="ExternalInput")
    relemb = nc.dram_tensor("relemb", [N_REL, C], F16, kind="ExternalInput")
    iotaf = nc.dram_tensor("iotaf", [P, P], F16, kind="ExternalInput")
    kg_tail = nc.dram_tensor("kg_tail", [P, EBLK * T], I32, kind="ExternalInput")
    kg_q = nc.dram_tensor("kg_q", [P, EBLK * T], I32, kind="ExternalInput")
    kg_hr = nc.dram_tensor("kg_hr", [P, EBLK * T], F16, kind="ExternalInput")
    kg_oh = nc.dram_tensor("kg_oh", [N_REL, EBLK * T * P], F16, kind="ExternalInput")
    ui_item = nc.dram_tensor("ui_item", [P, UBLK * TU], I32, kind="ExternalInput")
    ui_ur = nc.dram_tensor("ui_ur", [P, UBLK * TU], F16, kind="ExternalInput")
    ui_w = nc.dram_tensor("ui_w", [P, UBLK * TU], F16, kind="ExternalInput")

    entq = nc.dram_tensor("entq", [NEP, C], F16, kind="Internal")
    ag_in = nc.dram_tensor("ag_in", [EB, C], F16, kind="Internal")
    entv1 = nc.dram_tensor("entv1", [NEP, C], F16, kind="Internal",
                           addr_space="Shared")

    out_eres = nc.dram_tensor("out_eres", [EB, C], F32, kind="ExternalOutput")
    out_ures = nc.dram_tensor("out_ures", [UB, C], F32, kind="ExternalOutput")

    ds = bass.ds

    with tile.TileContext(nc) as tc, ExitStack() as ctx:
        cst = ctx.enter_context(tc.tile_pool(name="cst", bufs=1))
        sb = ctx.enter_context(tc.tile_pool(name="sb", bufs=3))
        gp = ctx.enter_context(tc.tile_pool(name="gp", bufs=2))
        ps = ctx.enter_context(tc.tile_pool(name="ps", bufs=2, space="PSUM"))
        psq = ctx.enter_context(tc.tile_pool(name="psq", bufs=4, space="PSUM"))

        wq_sb = cst.tile([C, C], F16)
        nc.sync.dma_start(out=wq_sb[:], in_=wq[:, :])
        rel_sb = cst.tile([N_REL, C], F16)
        nc.sync.dma_start(out=rel_sb[:], in_=relemb[:, :])
        iota_raw = cst.tile([P, P], F16)
        nc.sync.dma_start(out=iota_raw[:], in_=iotaf[:, :])
        iota_sb = cst.tile([P, P], F16)
        nc.vector.tensor_copy(out=iota_sb[:], in_=iota_raw[:])
        ident = cst.tile([P, P], F32)
        make_identity(nc, ident[:])

        # ---------------- entq = src @ W_Q (full table, local) --------------
        def entq_pass(src_T=None, src_rows=None):
            # src_T: [C, NEP] channel-major source (no transpose needed)
            # src_rows: [NEP, C] row-major source (transpose on PE)
            def body(iv):
                if src_T is not None:
                    lt = sb.tile([C, P], F16, tag="eqlt")
                    nc.sync.dma_start(out=lt[:], in_=src_T[:, ds(iv * P, P)])
                else:
                    ev = sb.tile([P, C], F16, tag="eqev")
                    nc.sync.dma_start(out=ev[:], in_=src_rows[ds(iv * P, P), :])
                    tp = psq.tile([P, C], F32, tag="mm")
                    nc.tensor.transpose(out=tp[:], in_=ev[:], identity=ident[:])
                    lt = sb.tile([C, P], F16, tag="eqlt")
                    nc.scalar.activation(lt[:], tp[:], Act.Copy)
                qp = psq.tile([P, C], F32, tag="mm")
                nc.tensor.matmul(out=qp[:], lhsT=lt[:], rhs=wq_sb[:],
                                 start=True, stop=True)
                qs = sb.tile([P, C], F16, tag="eqqs")
                nc.vector.tensor_copy(out=qs[:], in_=qp[:])
                nc.sync.dma_start(out=entq[ds(iv * P, P), :], in_=qs[:])
            tc.For_i_unrolled(0, NEP // P, 1, body, max_unroll=8)

        # ---------------- KG pass (one hop) ---------------------------------
        def kg_pass(hop, ventry):
            def body(iv):
                st_tail = sb.tile([P, T], I32, tag="sttail")
                nc.sync.dma_start(out=st_tail[:], in_=kg_tail[:, ds(iv * T, T)])
                st_q = sb.tile([P, T], I32, tag="stq")
                nc.sync.dma_start(out=st_q[:], in_=kg_q[:, ds(iv * T, T)])
                st_hr = sb.tile([P, T], F16, tag="sthr")
                nc.sync.dma_start(out=st_hr[:], in_=kg_hr[:, ds(iv * T, T)])
                st_oh = sb.tile([N_REL, T * P], F16, tag="stoh")
                nc.sync.dma_start(out=st_oh[:],
                                  in_=kg_oh[:, ds(iv * T * P, T * P)])

                tailv = gp.tile([P, T * C], F16, tag="tailv")
                tailq = gp.tile([P, T * C], F16, tag="tailq")
                qg = gp.tile([P, T * C], F16, tag="qg")
                for t in range(T):
                    nc.gpsimd.indirect_dma_start(
                        out=tailv[:, t * C:(t + 1) * C],
                        out_offset=None, in_=ventry[:],
                        in_offset=bass.IndirectOffsetOnAxis(
                            ap=st_tail[:, t:t + 1], axis=0))
                    nc.gpsimd.indirect_dma_start(
                        out=tailq[:, t * C:(t + 1) * C],
                        out_offset=None, in_=entq[:],
                        in_offset=bass.IndirectOffsetOnAxis(
                            ap=st_tail[:, t:t + 1], axis=0))
                    nc.gpsimd.indirect_dma_start(
                        out=qg[:, t * C:(t + 1) * C],
                        out_offset=None, in_=entq[:],
                        in_offset=bass.IndirectOffsetOnAxis(
                            ap=st_q[:, t:t + 1], axis=0))

                aggp = ps.tile([P, 130], F32, tag="agg")
                for t in range(T):
                    relp = ps.tile([P, C], F32, tag="relp")
                    nc.tensor.matmul(out=relp[:],
                                     lhsT=st_oh[:, t * P:(t + 1) * P],
                                     rhs=rel_sb[:], start=True, stop=True)
                    rel16 = sb.tile([P, C], F16, tag="rel16")
                    nc.scalar.activation(rel16[:], relp[:], Act.Copy)
                    val = sb.tile([P, C], F16, tag="val")
                    nc.vector.tensor_tensor(out=val[:],
                                            in0=tailv[:, t * C:(t + 1) * C],
                                            in1=rel16[:], op=Alu.mult)
                    kk = sb.tile([P, C], F16, tag="kk")
                    nc.vector.tensor_tensor(out=kk[:],
                                            in0=tailq[:, t * C:(t + 1) * C],
                                            in1=rel16[:], op=Alu.mult)
                    uu = sb.tile([P, C], F16, tag="uu")
                    nc.vector.tensor_tensor(out=uu[:],
                                            in0=qg[:, t * C:(t + 1) * C],
                                            in1=kk[:], op=Alu.mult)
                    sc = sb.tile([P, 2], F32, tag="sc")
                    nc.vector.tensor_reduce(
                        out=sc[:],
                        in_=uu[:].rearrange("p (h d) -> p h d", h=H),
                        axis=mybir.AxisListType.X, op=Alu.add)
                    rhs_t = sb.tile([P, 130], F16, tag="rhst")
                    nc.scalar.activation(rhs_t[:, 128:130], sc[:], Act.Exp,
                                         scale=0.125)
                    for h in range(H):
                        nc.vector.tensor_scalar_mul(
                            rhs_t[:, h * DK:(h + 1) * DK],
                            val[:, h * DK:(h + 1) * DK],
                            rhs_t[:, 128 + h:129 + h])
                    S = sb.tile([P, P], F16, tag="S")
                    nc.vector.tensor_tensor(
                        out=S[:],
                        in0=st_hr[:, t:t + 1].to_broadcast([P, P]),
                        in1=iota_sb[:, :], op=Alu.is_equal)
                    nc.tensor.matmul(out=aggp[:], lhsT=S[:], rhs=rhs_t[:],
                                     start=(t == 0), stop=(t == T - 1))

                esum = sb.tile([P, 2], F32, tag="esum")
                nc.vector.tensor_scalar_max(esum[:], aggp[:, 128:130], 1e-16)
                inv = sb.tile([P, 2], F32, tag="inv")
                nc.vector.reciprocal(inv[:], esum[:])
                ea = sb.tile([P, C], F32, tag="ea")
                for h in range(H):
                    nc.scalar.activation(ea[:, h * DK:(h + 1) * DK],
                                         aggp[:, h * DK:(h + 1) * DK],
                                         Act.Copy, scale=inv[:, h:h + 1])
                ssq = sb.tile([P, 1], F32, tag="ssq")
                scr2 = sb.tile([P, C], F32, tag="scr2")
                nc.vector.tensor_tensor(out=scr2[:], in0=ea[:], in1=ea[:],
                                        op=Alu.mult)
                nc.vector.tensor_reduce(out=ssq[:], in_=scr2[:],
                                        axis=mybir.AxisListType.X, op=Alu.add)
                nrm = sb.tile([P, 1], F32, tag="nrm")
                nc.scalar.activation(nrm[:], ssq[:], Act.Sqrt)
                nc.vector.tensor_scalar_max(nrm[:], nrm[:], 1e-12)
                invn = sb.tile([P, 1], F32, tag="invn")
                nc.vector.reciprocal(invn[:], nrm[:])
                e1 = sb.tile([P, C], F32, tag="e1")
                nc.scalar.activation(e1[:], ea[:], Act.Copy,
                                     scale=invn[:, 0:1])
                if hop == 0:
                    nc.gpsimd.dma_start(out=ag_in[ds(iv * P, P), :], in_=e1[:])
                resold = sb.tile([P, C], F32, tag="resold")
                src = e0_blk if hop == 0 else out_eres
                nc.sync.dma_start(out=resold[:], in_=src[ds(iv * P, P), :])
                resn = sb.tile([P, C], F32, tag="resn")
                nc.vector.tensor_add(out=resn[:], in0=resold[:], in1=e1[:])
                nc.sync.dma_start(out=out_eres[ds(iv * P, P), :], in_=resn[:])
            tc.For_i_unrolled(0, EBLK, 1, body, max_unroll=4)

        # ---------------- UI pass (one hop) ---------------------------------
        def ui_pass(hop, ventry):
            def body(iv):
                st_it = sb.tile([P, TU], I32, tag="stit")
                nc.sync.dma_start(out=st_it[:], in_=ui_item[:, ds(iv * TU, TU)])
                st_ur = sb.tile([P, TU], F16, tag="stur")
                nc.sync.dma_start(out=st_ur[:], in_=ui_ur[:, ds(iv * TU, TU)])
                st_w = sb.tile([P, TU], F16, tag="stw")
                nc.sync.dma_start(out=st_w[:], in_=ui_w[:, ds(iv * TU, TU)])

                itemv = gp.tile([P, TU * C], F16, tag="itemv")
                for t in range(TU):
                    nc.gpsimd.indirect_dma_start(
                        out=itemv[:, t * C:(t + 1) * C],
                        out_offset=None, in_=ventry[:],
                        in_offset=bass.IndirectOffsetOnAxis(
                            ap=st_it[:, t:t + 1], axis=0))

                aggp = ps.tile([P, 130], F32, tag="agg")
                for t in range(TU):
                    msg = sb.tile([P, C], F16, tag="umsg")
                    nc.vector.tensor_scalar_mul(
                        msg[:], itemv[:, t * C:(t + 1) * C], st_w[:, t:t + 1])
                    S = sb.tile([P, P], F16, tag="US")
                    nc.vector.tensor_tensor(
                        out=S[:],
                        in0=st_ur[:, t:t + 1].to_broadcast([P, P]),
                        in1=iota_sb[:, :], op=Alu.is_equal)
                    nc.tensor.matmul(out=aggp[:, :C], lhsT=S[:], rhs=msg[:],
                                     start=(t == 0), stop=(t == TU - 1))

                ua = sb.tile([P, C], F32, tag="ua")
                nc.vector.tensor_copy(out=ua[:], in_=aggp[:, :C])
                ssq = sb.tile([P, 1], F32, tag="ussq")
                scr2 = sb.tile([P, C], F32, tag="uscr2")
                nc.vector.tensor_tensor(out=scr2[:], in0=ua[:],
                                        in1=ua[:], op=Alu.mult)
                nc.vector.tensor_reduce(out=ssq[:], in_=scr2[:],
                                        axis=mybir.AxisListType.X, op=Alu.add)
                nrm = sb.tile([P, 1], F32, tag="unrm")
                nc.scalar.activation(nrm[:], ssq[:], Act.Sqrt)
                nc.vector.tensor_scalar_max(nrm[:], nrm[:], 1e-12)
                invn = sb.tile([P, 1], F32, tag="uinvn")
                nc.vector.reciprocal(invn[:], nrm[:])
                u1 = sb.tile([P, C], F32, tag="u1")
                nc.scalar.activation(u1[:], ua[:], Act.Copy,
                                     scale=invn[:, 0:1])
                resold = sb.tile([P, C], F32, tag="uresold")
                src = u0_blk if hop == 0 else out_ures
                nc.sync.dma_start(out=resold[:], in_=src[ds(iv * P, P), :])
                resn = sb.tile([P, C], F32, tag="uresn")
                nc.vector.tensor_add(out=resn[:], in0=resold[:], in1=u1[:])
                nc.sync.dma_start(out=out_ures[ds(iv * P, P), :], in_=resn[:])
            tc.For_i_unrolled(0, UBLK, 1, body, max_unroll=4)

        # ------------------------------ schedule -----------------------------
        entq_pass(src_T=e0T)
        kg_pass(0, entv0)
        ui_pass(0, entv0)
        nc.gpsimd.collective_compute(
            "AllGather", Alu.bypass,
            replica_groups=[list(range(CORES))],
            ins=[ag_in[:, :]], outs=[entv1[:, :]])
        entq_pass(src_rows=entv1)
        kg_pass(1, entv1)
        ui_pass(1, entv1)

    nc.compile()
    return nc


_PROG_CACHE = {}


def _get_program(T, TU):
    key = (T, TU)
    if key not in _PROG_CACHE:
        _PROG_CACHE[key] = build_program(T, TU)
    return _PROG_CACHE[key]


def _build_in_maps(inputs, kg_maps, ui_maps):
    entity_emb = np.asarray(inputs["entity_emb"], np.float32)
    user_emb = np.asarray(inputs["user_emb"], np.float32)
    relation_emb = np.asarray(inputs["relation_emb"], np.float32)
    W_Q = np.asarray(inputs["W_Q"], np.float32)

    entv0f = np.zeros((NEP, C), np.float32)
    entv0f[:N_ENT] = entity_emb
    entv0 = entv0f.astype(np.float16)
    e0T = np.ascontiguousarray(entv0.T)
    u0 = np.zeros((NUP, C), np.float32)
    u0[:N_USERS] = user_emb
    iotaf = np.tile(np.arange(P, dtype=np.float16)[None, :], (P, 1))

    in_maps = []
    for c in range(CORES):
        m = dict(
            entv0=entv0, e0T=e0T,
            e0_blk=np.ascontiguousarray(entv0f[c * EB:(c + 1) * EB]),
            u0_blk=np.ascontiguousarray(u0[c * UB:(c + 1) * UB]),
            wq=W_Q.astype(np.float16), relemb=relation_emb.astype(np.float16),
            iotaf=iotaf,
        )
        m.update(kg_maps[c])
        m.update(ui_maps[c])
        in_maps.append(m)
    return in_maps


def kernel(user_emb, entity_emb, edge_index, edge_type, inter_edge,
           inter_edge_w, relation_emb, W_Q):
    inputs = dict(user_emb=user_emb, entity_emb=entity_emb,
                  edge_index=edge_index, edge_type=edge_type,
                  inter_edge=inter_edge, inter_edge_w=inter_edge_w,
                  relation_emb=relation_emb, W_Q=W_Q)
    T, kg_maps = _prep_kg(np.asarray(edge_index), np.asarray(edge_type))
    TU, ui_maps = _prep_ui(np.asarray(inter_edge), np.asarray(inter_edge_w))
    nc = _get_program(T, TU)
    in_maps = _build_in_maps(inputs, kg_maps, ui_maps)
    res = run_bass_kernel_spmd(nc, in_maps, core_ids=list(range(CORES)))
    eres = np.concatenate([r["out_eres"] for r in res.results], 0)[:N_ENT]
    ures = np.concatenate([r["out_ures"] for r in res.results], 0)[:N_USERS]
    return eres, ures


# revision 11
# speedup vs baseline: 1.2866x; 1.2866x over previous
"""AttnHGCN 2-hop GNN message passing on 8 Trainium2 NeuronCores.

Strategy (edge/graph parallelism):
  - KG edges are bucketed by head (destination) node. Each core owns a
    contiguous range of 25088 entity rows (196 blocks of 128) and receives
    exactly the edges whose head falls in its range.  Within a core, edges
    are grouped per 128-node block and padded to T tiles of 128 edges.
  - Per edge tile: indirect-DMA gathers of entity rows, one-hot matmul for
    relation embedding lookup, fused multiply+reduce for attention logits,
    exp on the scalar engine, and a PE matmul with a data-built 0/1
    selection matrix that performs the per-node segment sum of both the
    exp-weighted messages and the softmax denominators in one PSUM
    accumulation.
  - Per block: divide by the denominator, l2-normalize, accumulate the
    residual.  User/item edges follow the same scheme (no softmax).
  - Hop boundary: AllGather of the normalized entity blocks, then a full
    local pass applying W_Q to rebuild the per-node query table.
"""
import sys
sys.path.insert(0, "/opt/trn_rl_repo")

import math
from contextlib import ExitStack
import numpy as np

from concourse import bass, bacc, mybir, tile
from concourse.bass_utils import run_bass_kernel_spmd
from concourse.masks import make_identity

P = 128
C = 128
H = 2
DK = 64
N_USERS = 100_000
N_ENT = 200_000
N_REL = 9            # used relations (edge_type-1 in [0, 9))
E_KG = 1_000_000
E_UI = 1_000_000
CORES = 8

EBLK = 196           # entity 128-blocks per core
EB = EBLK * P        # 25088 entity rows per core
NEP = CORES * EB     # 200704 padded entity rows
UBLK = 98            # user 128-blocks per core
UB = UBLK * P        # 12544 user rows per core
NUP = CORES * UB

F32 = mybir.dt.float32
F16 = mybir.dt.float16
I32 = mybir.dt.int32
Alu = mybir.AluOpType
Act = mybir.ActivationFunctionType


# ----------------------------------------------------------------- host prep

def _bucketize(dest, ncores, rows_per_core, blocks_per_core):
    """Return (core, global_block, slot_rank, order) for each edge sorted by
    global block, plus per-block counts."""
    core = dest // rows_per_core
    blk = (dest % rows_per_core) // P
    gb = core * blocks_per_core + blk
    order = np.argsort(gb, kind="stable")
    gbs = gb[order]
    nblocks = ncores * blocks_per_core
    counts = np.bincount(gbs, minlength=nblocks)
    starts = np.concatenate([[0], np.cumsum(counts)[:-1]])
    rank = np.arange(len(dest)) - starts[gbs]
    return order, gbs, rank, counts


def _prep_kg(edge_index, edge_type):
    head = np.asarray(edge_index[0], dtype=np.int64)
    tail = np.asarray(edge_index[1], dtype=np.int64)
    et = np.asarray(edge_type, dtype=np.int64) - 1
    order, gbs, rank, counts = _bucketize(head, CORES, EB, EBLK)
    T = max(1, int(math.ceil(counts.max() / P)))
    nslot_blk = T * P
    nslots = CORES * EBLK * nslot_blk
    g_tail = np.zeros(nslots, np.int32)
    g_q = np.zeros(nslots, np.int32)
    g_hr = np.full(nslots, -1.0, np.float32)
    g_oh = np.zeros((N_REL, nslots), np.float32)
    slot = gbs * nslot_blk + rank
    g_tail[slot] = tail[order]
    g_q[slot] = head[order]
    g_hr[slot] = (head[order] % P).astype(np.float32)
    g_oh[et[order], slot] = 1.0
    per_core = EBLK * nslot_blk
    outs = []
    for c in range(CORES):
        sl = slice(c * per_core, (c + 1) * per_core)
        outs.append(dict(
            kg_tail=g_tail[sl].reshape(EBLK * T, P).T.copy(),
            kg_q=g_q[sl].reshape(EBLK * T, P).T.copy(),
            kg_hr=g_hr[sl].reshape(EBLK * T, P).T.astype(np.float16),
            kg_oh=np.ascontiguousarray(g_oh[:, sl]).astype(np.float16),
        ))
    return T, outs


def _prep_ui(inter_edge, inter_w):
    user = np.asarray(inter_edge[0], dtype=np.int64)
    item = np.asarray(inter_edge[1], dtype=np.int64)
    w = np.asarray(inter_w, dtype=np.float32)
    order, gbs, rank, counts = _bucketize(user, CORES, UB, UBLK)
    T = max(1, int(math.ceil(counts.max() / P)))
    nslot_blk = T * P
    nslots = CORES * UBLK * nslot_blk
    g_item = np.zeros(nslots, np.int32)
    g_ur = np.full(nslots, -1.0, np.float32)
    g_w = np.zeros(nslots, np.float32)
    slot = gbs * nslot_blk + rank
    g_item[slot] = item[order]
    g_ur[slot] = (user[order] % P).astype(np.float32)
    g_w[slot] = w[order]
    per_core = UBLK * nslot_blk
    outs = []
    for c in range(CORES):
        sl = slice(c * per_core, (c + 1) * per_core)
        outs.append(dict(
            ui_item=g_item[sl].reshape(UBLK * T, P).T.copy(),
            ui_ur=g_ur[sl].reshape(UBLK * T, P).T.astype(np.float16),
            ui_w=g_w[sl].reshape(UBLK * T, P).T.copy(),
        ))
    return T, outs


# ------------------------------------------------------------- bass program

def build_program(T, TU):
    nc = bacc.Bacc(num_devices=CORES)

    entv0 = nc.dram_tensor("entv0", [NEP, C], F16, kind="ExternalInput")
    e0T = nc.dram_tensor("e0T", [C, NEP], F16, kind="ExternalInput")
    e0_blk = nc.dram_tensor("e0_blk", [EB, C], F32, kind="ExternalInput")
    u0_blk = nc.dram_tensor("u0_blk", [UB, C], F32, kind="ExternalInput")
    wq = nc.dram_tensor("wq", [C, C], F16, kind="ExternalInput")
    relemb = nc.dram_tensor("relemb", [N_REL, C], F16, kind="ExternalInput")
    iotaf = nc.dram_tensor("iotaf", [P, P], F16, kind="ExternalInput")
    kg_tail = nc.dram_tensor("kg_tail", [P, EBLK * T], I32, kind="ExternalInput")
    kg_q = nc.dram_tensor("kg_q", [P, EBLK * T], I32, kind="ExternalInput")
    kg_hr = nc.dram_tensor("kg_hr", [P, EBLK * T], F16, kind="ExternalInput")
    kg_oh = nc.dram_tensor("kg_oh", [N_REL, EBLK * T * P], F16, kind="ExternalInput")
    ui_item = nc.dram_tensor("ui_item", [P, UBLK * TU], I32, kind="ExternalInput")
    ui_ur = nc.dram_tensor("ui_ur", [P, UBLK * TU], F16, kind="ExternalInput")
    ui_w = nc.dram_tensor("ui_w", [P, UBLK * TU], F32, kind="ExternalInput")

    entq = nc.dram_tensor("entq", [NEP, C], F16, kind="Internal")
    ag_in = nc.dram_tensor("ag_in", [EB, C], F16, kind="Internal")
    entv1 = nc.dram_tensor("entv1", [NEP, C], F16, kind="Internal",
                           addr_space="Shared")

    out_eres = nc.dram_tensor("out_eres", [EB, C], F32, kind="ExternalOutput")
    out_ures = nc.dram_tensor("out_ures", [UB, C], F32, kind="ExternalOutput")

    ds = bass.ds

    with tile.TileContext(nc) as tc, ExitStack() as ctx:
        cst = ctx.enter_context(tc.tile_pool(name="cst", bufs=1))
        sb = ctx.enter_context(tc.tile_pool(name="sb", bufs=3))
        gp = ctx.enter_context(tc.tile_pool(name="gp", bufs=2))
        ps = ctx.enter_context(tc.tile_pool(name="ps", bufs=2, space="PSUM"))
        psq = ctx.enter_context(tc.tile_pool(name="psq", bufs=4, space="PSUM"))

        wq_sb = cst.tile([C, C], F16)
        nc.sync.dma_start(out=wq_sb[:], in_=wq[:, :])
        rel_sb = cst.tile([N_REL, C], F16)
        nc.sync.dma_start(out=rel_sb[:], in_=relemb[:, :])
        iota_raw = cst.tile([P, P], F16)
        nc.sync.dma_start(out=iota_raw[:], in_=iotaf[:, :])
        iota_sb = cst.tile([P, P], F16)
        nc.vector.tensor_copy(out=iota_sb[:], in_=iota_raw[:])
        ident = cst.tile([P, P], F32)
        make_identity(nc, ident[:])

        # ---------------- entq = src @ W_Q (full table, local) --------------
        def entq_pass(src_T=None, src_rows=None):
            # src_T: [C, NEP] channel-major source (no transpose needed)
            # src_rows: [NEP, C] row-major source (transpose on PE)
            def body(iv):
                if src_T is not None:
                    lt = sb.tile([C, P], F16, tag="eqlt")
                    nc.sync.dma_start(out=lt[:], in_=src_T[:, ds(iv * P, P)])
                else:
                    ev = sb.tile([P, C], F16, tag="eqev")
                    nc.sync.dma_start(out=ev[:], in_=src_rows[ds(iv * P, P), :])
                    tp = psq.tile([P, C], F32, tag="mm")
                    nc.tensor.transpose(out=tp[:], in_=ev[:], identity=ident[:])
                    lt = sb.tile([C, P], F16, tag="eqlt")
                    nc.scalar.activation(lt[:], tp[:], Act.Copy)
                qp = psq.tile([P, C], F32, tag="mm")
                nc.tensor.matmul(out=qp[:], lhsT=lt[:], rhs=wq_sb[:],
                                 start=True, stop=True)
                qs = sb.tile([P, C], F16, tag="eqqs")
                nc.vector.tensor_copy(out=qs[:], in_=qp[:])
                nc.sync.dma_start(out=entq[ds(iv * P, P), :], in_=qs[:])
            tc.For_i_unrolled(0, NEP // P, 1, body, max_unroll=8)

        # ---------------- KG pass (one hop) ---------------------------------
        def kg_pass(hop, ventry):
            def body(iv):
                st_tail = sb.tile([P, T], I32, tag="sttail")
                nc.sync.dma_start(out=st_tail[:], in_=kg_tail[:, ds(iv * T, T)])
                st_q = sb.tile([P, T], I32, tag="stq")
                nc.sync.dma_start(out=st_q[:], in_=kg_q[:, ds(iv * T, T)])
                st_hr = sb.tile([P, T], F16, tag="sthr")
                nc.sync.dma_start(out=st_hr[:], in_=kg_hr[:, ds(iv * T, T)])
                st_oh = sb.tile([N_REL, T * P], F16, tag="stoh")
                nc.sync.dma_start(out=st_oh[:],
                                  in_=kg_oh[:, ds(iv * T * P, T * P)])

                tailv = gp.tile([P, T * C], F16, tag="tailv")
                tailq = gp.tile([P, T * C], F16, tag="tailq")
                qg = gp.tile([P, T * C], F16, tag="qg")
                for t in range(T):
                    nc.gpsimd.indirect_dma_start(
                        out=tailv[:, t * C:(t + 1) * C],
                        out_offset=None, in_=ventry[:],
                        in_offset=bass.IndirectOffsetOnAxis(
                            ap=st_tail[:, t:t + 1], axis=0))
                    nc.gpsimd.indirect_dma_start(
                        out=tailq[:, t * C:(t + 1) * C],
                        out_offset=None, in_=entq[:],
                        in_offset=bass.IndirectOffsetOnAxis(
                            ap=st_tail[:, t:t + 1], axis=0))
                    nc.gpsimd.indirect_dma_start(
                        out=qg[:, t * C:(t + 1) * C],
                        out_offset=None, in_=entq[:],
                        in_offset=bass.IndirectOffsetOnAxis(
                            ap=st_q[:, t:t + 1], axis=0))

                aggp = ps.tile([P, 130], F32, tag="agg")
                for t in range(T):
                    relp = ps.tile([P, C], F32, tag="relp")
                    nc.tensor.matmul(out=relp[:],
                                     lhsT=st_oh[:, t * P:(t + 1) * P],
                                     rhs=rel_sb[:], start=True, stop=True)
                    rel16 = sb.tile([P, C], F16, tag="rel16")
                    nc.scalar.activation(rel16[:], relp[:], Act.Copy)
                    val = sb.tile([P, C], F16, tag="val")
                    nc.vector.tensor_tensor(out=val[:],
                                            in0=tailv[:, t * C:(t + 1) * C],
                                            in1=rel16[:], op=Alu.mult)
                    kk = sb.tile([P, C], F16, tag="kk")
                    nc.vector.tensor_tensor(out=kk[:],
                                            in0=tailq[:, t * C:(t + 1) * C],
                                            in1=rel16[:], op=Alu.mult)
                    uu = sb.tile([P, C], F16, tag="uu")
                    nc.vector.tensor_tensor(out=uu[:],
                                            in0=qg[:, t * C:(t + 1) * C],
                                            in1=kk[:], op=Alu.mult)
                    sc = sb.tile([P, 2], F32, tag="sc")
                    nc.vector.tensor_reduce(
                        out=sc[:],
                        in_=uu[:].rearrange("p (h d) -> p h d", h=H),
                        axis=mybir.AxisListType.X, op=Alu.add)
                    rhs_t = sb.tile([P, 130], F16, tag="rhst")
                    ex = sb.tile([P, 2], F32, tag="ex")
                    nc.scalar.activation(ex[:], sc[:], Act.Exp, scale=0.125)
                    nc.vector.tensor_copy(out=rhs_t[:, 128:130], in_=ex[:])
                    for h in range(H):
                        nc.vector.tensor_scalar_mul(
                            rhs_t[:, h * DK:(h + 1) * DK],
                            val[:, h * DK:(h + 1) * DK],
                            ex[:, h:h + 1])
                    S = sb.tile([P, P], F16, tag="S")
                    nc.vector.tensor_tensor(
                        out=S[:],
                        in0=st_hr[:, t:t + 1].to_broadcast([P, P]),
                        in1=iota_sb[:, :], op=Alu.is_equal)
                    nc.tensor.matmul(out=aggp[:], lhsT=S[:], rhs=rhs_t[:],
                                     start=(t == 0), stop=(t == T - 1))

                esum = sb.tile([P, 2], F32, tag="esum")
                nc.vector.tensor_scalar_max(esum[:], aggp[:, 128:130], 1e-16)
                inv = sb.tile([P, 2], F32, tag="inv")
                nc.vector.reciprocal(inv[:], esum[:])
                ea = sb.tile([P, C], F32, tag="ea")
                for h in range(H):
                    nc.scalar.activation(ea[:, h * DK:(h + 1) * DK],
                                         aggp[:, h * DK:(h + 1) * DK],
                                         Act.Copy, scale=inv[:, h:h + 1])
                ssq = sb.tile([P, 1], F32, tag="ssq")
                scr2 = sb.tile([P, C], F32, tag="scr2")
                nc.vector.tensor_tensor(out=scr2[:], in0=ea[:], in1=ea[:],
                                        op=Alu.mult)
                nc.vector.tensor_reduce(out=ssq[:], in_=scr2[:],
                                        axis=mybir.AxisListType.X, op=Alu.add)
                nrm = sb.tile([P, 1], F32, tag="nrm")
                nc.scalar.activation(nrm[:], ssq[:], Act.Sqrt)
                nc.vector.tensor_scalar_max(nrm[:], nrm[:], 1e-12)
                invn = sb.tile([P, 1], F32, tag="invn")
                nc.vector.reciprocal(invn[:], nrm[:])
                e1 = sb.tile([P, C], F32, tag="e1")
                nc.scalar.activation(e1[:], ea[:], Act.Copy,
                                     scale=invn[:, 0:1])
                if hop == 0:
                    nc.gpsimd.dma_start(out=ag_in[ds(iv * P, P), :], in_=e1[:])
                resold = sb.tile([P, C], F32, tag="resold")
                src = e0_blk if hop == 0 else out_eres
                nc.sync.dma_start(out=resold[:], in_=src[ds(iv * P, P), :])
                resn = sb.tile([P, C], F32, tag="resn")
                nc.vector.tensor_add(out=resn[:], in0=resold[:], in1=e1[:])
                nc.sync.dma_start(out=out_eres[ds(iv * P, P), :], in_=resn[:])
            tc.For_i_unrolled(0, EBLK, 1, body, max_unroll=4)

        # ---------------- UI pass (one hop) ---------------------------------
        def ui_pass(hop, ventry):
            def body(iv):
                st_it = sb.tile([P, TU], I32, tag="stit")
                nc.sync.dma_start(out=st_it[:], in_=ui_item[:, ds(iv * TU, TU)])
                st_ur = sb.tile([P, TU], F16, tag="stur")
                nc.sync.dma_start(out=st_ur[:], in_=ui_ur[:, ds(iv * TU, TU)])
                st_w = sb.tile([P, TU], F32, tag="stw")
                nc.sync.dma_start(out=st_w[:], in_=ui_w[:, ds(iv * TU, TU)])

                itemv = gp.tile([P, TU * C], F16, tag="itemv")
                for t in range(TU):
                    nc.gpsimd.indirect_dma_start(
                        out=itemv[:, t * C:(t + 1) * C],
                        out_offset=None, in_=ventry[:],
                        in_offset=bass.IndirectOffsetOnAxis(
                            ap=st_it[:, t:t + 1], axis=0))

                aggp = ps.tile([P, 130], F32, tag="agg")
                for t in range(TU):
                    msg = sb.tile([P, C], F16, tag="umsg")
                    nc.vector.tensor_scalar_mul(
                        msg[:], itemv[:, t * C:(t + 1) * C], st_w[:, t:t + 1])
                    S = sb.tile([P, P], F16, tag="US")
                    nc.vector.tensor_tensor(
                        out=S[:],
                        in0=st_ur[:, t:t + 1].to_broadcast([P, P]),
                        in1=iota_sb[:, :], op=Alu.is_equal)
                    nc.tensor.matmul(out=aggp[:, :C], lhsT=S[:], rhs=msg[:],
                                     start=(t == 0), stop=(t == TU - 1))

                ua = sb.tile([P, C], F32, tag="ua")
                nc.vector.tensor_copy(out=ua[:], in_=aggp[:, :C])
                ssq = sb.tile([P, 1], F32, tag="ussq")
                scr2 = sb.tile([P, C], F32, tag="uscr2")
                nc.vector.tensor_tensor(out=scr2[:], in0=ua[:],
                                        in1=ua[:], op=Alu.mult)
                nc.vector.tensor_reduce(out=ssq[:], in_=scr2[:],
                                        axis=mybir.AxisListType.X, op=Alu.add)
                nrm = sb.tile([P, 1], F32, tag="unrm")
                nc.scalar.activation(nrm[:], ssq[:], Act.Sqrt)
                nc.vector.tensor_scalar_max(nrm[:], nrm[:], 1e-12)
                invn = sb.tile([P, 1], F32, tag="uinvn")
                nc.vector.reciprocal(invn[:], nrm[:])
                u1 = sb.tile([P, C], F32, tag="u1")
                nc.scalar.activation(u1[:], ua[:], Act.Copy,
                                     scale=invn[:, 0:1])
                resold = sb.tile([P, C], F32, tag="uresold")
                src = u0_blk if hop == 0 else out_ures
                nc.sync.dma_start(out=resold[:], in_=src[ds(iv * P, P), :])
                resn = sb.tile([P, C], F32, tag="uresn")
                nc.vector.tensor_add(out=resn[:], in0=resold[:], in1=u1[:])
                nc.sync.dma_start(out=out_ures[ds(iv * P, P), :], in_=resn[:])
            tc.For_i_unrolled(0, UBLK, 1, body, max_unroll=4)

        # ------------------------------ schedule -----------------------------
        entq_pass(src_T=e0T)
        kg_pass(0, entv0)
        ui_pass(0, entv0)
        nc.gpsimd.collective_compute(
            "AllGather", Alu.bypass,
            replica_groups=[list(range(CORES))],
            ins=[ag_in[:, :]], outs=[entv1[:, :]])
        entq_pass(src_rows=entv1)
        kg_pass(1, entv1)
        ui_pass(1, entv1)

    nc.compile()
    return nc


_PROG_CACHE = {}


def _get_program(T, TU):
    key = (T, TU)
    if key not in _PROG_CACHE:
        _PROG_CACHE[key] = build_program(T, TU)
    return _PROG_CACHE[key]


def _build_in_maps(inputs, kg_maps, ui_maps):
    entity_emb = np.asarray(inputs["entity_emb"], np.float32)
    user_emb = np.asarray(inputs["user_emb"], np.float32)
    relation_emb = np.asarray(inputs["relation_emb"], np.float32)
    W_Q = np.asarray(inputs["W_Q"], np.float32)

    entv0f = np.zeros((NEP, C), np.float32)
    entv0f[:N_ENT] = entity_emb
    entv0 = entv0f.astype(np.float16)
    e0T = np.ascontiguousarray(entv0.T)
    u0 = np.zeros((NUP, C), np.float32)
    u0[:N_USERS] = user_emb
    iotaf = np.tile(np.arange(P, dtype=np.float16)[None, :], (P, 1))

    in_maps = []
    for c in range(CORES):
        m = dict(
            entv0=entv0, e0T=e0T,
            e0_blk=np.ascontiguousarray(entv0f[c * EB:(c + 1) * EB]),
            u0_blk=np.ascontiguousarray(u0[c * UB:(c + 1) * UB]),
            wq=W_Q.astype(np.float16), relemb=relation_emb.astype(np.float16),
            iotaf=iotaf,
        )
        m.update(kg_maps[c])
        m.update(ui_maps[c])
        in_maps.append(m)
    return in_maps


def kernel(user_emb, entity_emb, edge_index, edge_type, inter_edge,
           inter_edge_w, relation_emb, W_Q):
    inputs = dict(user_emb=user_emb, entity_emb=entity_emb,
                  edge_index=edge_index, edge_type=edge_type,
                  inter_edge=inter_edge, inter_edge_w=inter_edge_w,
                  relation_emb=relation_emb, W_Q=W_Q)
    T, kg_maps = _prep_kg(np.asarray(edge_index), np.asarray(edge_type))
    TU, ui_maps = _prep_ui(np.asarray(inter_edge), np.asarray(inter_edge_w))
    nc = _get_program(T, TU)
    in_maps = _build_in_maps(inputs, kg_maps, ui_maps)
    res = run_bass_kernel_spmd(nc, in_maps, core_ids=list(range(CORES)))
    eres = np.concatenate([r["out_eres"] for r in res.results], 0)[:N_ENT]
    ures = np.concatenate([r["out_ures"] for r in res.results], 0)[:N_USERS]
    return eres, ures
